# revision 45
# baseline (speedup 1.0000x reference)
"""MoE per-sample expert conv3x3 (320->320, 64x64, B=16, 5 experts) on 8 trn2 cores.

Strategy: data-parallel over batch (2 samples/core). Host gathers each
sample's expert weights (weights[class_id]), transposes them to lhsT layout
[tap, CIN, COUT], and zero-pads features to 66x66 so the conv becomes 9
shifted matmuls accumulating in PSUM (no boundary handling on device).

float32r matmuls: full fp32 I/O, 11-bit mantissa multiplies, 4x the fp32
rate on the PE at N>=256.

CIN=320 splits into chunks (128,128,64). The 64-row remainder chunk would
leave half the PE array rows idle, so its kw=0/kw=1 tap pairs are packed
via tile_position row tiling: rows 0:64 of the array run tap (kh,0) while
rows 64:128 concurrently run tap (kh,1), whose weights are overlaid in the
upper partition half of the chunk-2 weight tile and whose features come
from a +1-shifted duplicate in the upper half of the chunk-2 feature tile.
The row-B results accumulate in a second PSUM bank per n-tile and are
folded in during the bias epilogue.

DMA: features on the SP HWDGE ring (nc.sync), weights/bias/outputs on the
ACT ring (nc.scalar) so the two input streams load in parallel at startup.
"""

import time

import numpy as np

import concourse.bass as bass
import concourse.mybir as mybir
import concourse.tile as tile
from concourse import bacc
from concourse.bass_utils import run_bass_kernel_spmd

B = 16
NCORES = 8
S = B // NCORES          # samples per core
CIN = 320
COUT = 320
H = W = 64
KK = 3
HP = WP = H + 2          # padded spatial
NPIX = H * W             # 4096
NT = 512                 # output free-dim tile: 8 rows x 64 cols
ROWS_PER_NT = NT // W    # 8
NTILES = NPIX // NT      # 8
NG = 2                   # n-tiles per PSUM group
KCH = [(0, 128), (128, 128), (256, 64)]   # CIN chunks (k0, ksize)
MCH = [(0, 128), (128, 128), (256, 64)]   # COUT chunks (m0, msize)

DT_MM = mybir.dt.float32r   # matmul compute dtype (fp32 storage, fast path)
PACK_K = True               # row-pack the CIN=64 remainder chunk tap pairs


def build_nc():
    nc = bacc.Bacc(None, target_bir_lowering=False)
    xpad = nc.dram_tensor("xpad", [S, CIN, HP * WP], DT_MM,
                          kind="ExternalInput")
    wt = nc.dram_tensor("wt", [S, KK * KK, CIN, COUT], DT_MM,
                        kind="ExternalInput")
    bias = nc.dram_tensor("bias", [S, COUT], mybir.dt.float32,
                          kind="ExternalInput")
    y = nc.dram_tensor("y", [S, COUT, NPIX], mybir.dt.float32,
                       kind="ExternalOutput")

    with tile.TileContext(nc) as tc:
        with (
            tc.tile_pool(name="wpool", bufs=1) as wpool,
            tc.tile_pool(name="xpool", bufs=2) as xpool,
            tc.tile_pool(name="bpool", bufs=1) as bpool,
            tc.tile_pool(name="opool", bufs=8) as opool,
            tc.tile_pool(name="psum", bufs=2, space="PSUM") as psum_pool,
        ):
            btile = bpool.tile([128, S * 3], mybir.dt.float32, name="btile",
                               tag="btile")

            wts_all = {}
            xts_all = {}
            for s in range(S):
                # features first on the SP ring: the first matmuls need x
                # x in two row-pieces per chunk, all top pieces first, so
                # the first n-tile groups' matmuls unblock on every chunk
                # as early as possible (Tile tracks subtile deps)
                # interleave per-chunk: x top piece, then that chunk's
                # weights, so the serial DMA pool delivers data in the
                # order the PE consumes it (x on SP ring, wt on ACT ring)
                cut = 34 * WP
                xts = []
                wts = []
                for ci, (c0, cs) in enumerate(KCH):
                    xt = xpool.tile([128, HP * WP], DT_MM,
                                    name=f"x_{s}_{ci}", tag=f"x{ci}")
                    nc.sync.dma_start(out=xt[:cs, :cut],
                                      in_=xpad[s, c0 : c0 + cs, :cut])
                    if PACK_K and ci == 2:
                        # upper half: same 64 channels shifted by +1 column,
                        # so row-tile B at window(kh,0) reads tap (kh,1) data
                        nc.sync.dma_start(
                            out=xt[64:128, 0 : cut],
                            in_=xpad[s, c0 : c0 + cs, 1 : cut + 1])
                    xts.append(xt)

                    wti = wpool.tile([128, KK * KK * COUT], DT_MM,
                                     name=f"wt_{s}_{ci}", tag=f"wt_{s}_{ci}")
                    dst = wti[:cs].rearrange("c (t o) -> c t o", o=COUT)
                    src = wt[s].rearrange("t c o -> c t o")[c0 : c0 + cs]
                    # tap 0 first: unblocks the first accumulation slot
                    nc.scalar.dma_start(out=dst[:, 0:1], in_=src[:, 0:1])
                    nc.scalar.dma_start(out=dst[:, 1:], in_=src[:, 1:])
                    if PACK_K and ci == 2:
                        # upper half, at tap-(kh,0) columns: tap (kh,1)
                        # weights for the row-B halves of the k-pairs
                        for kh in range(KK):
                            nc.scalar.dma_start(
                                out=wti[64 : 64 + cs,
                                        (kh * KK) * COUT :
                                        (kh * KK) * COUT + COUT],
                                in_=wt[s, kh * KK + 1, c0 : c0 + cs],
                            )
                    wts.append(wti)
                for ci, (c0, cs) in enumerate(KCH):
                    nc.sync.dma_start(out=xts[ci][:cs, cut:],
                                      in_=xpad[s, c0 : c0 + cs, cut:])
                    if PACK_K and ci == 2:
                        nc.sync.dma_start(
                            out=xts[ci][64:128, cut : HP * WP - 1],
                            in_=xpad[s, c0 : c0 + cs, cut + 1 : HP * WP])
                xts_all[s] = xts
                wts_all[s] = wts

            for s in range(S):
                for mi, (m0, ms) in enumerate(MCH):
                    nc.scalar.dma_start(
                        out=btile[:ms, s * 3 + mi : s * 3 + mi + 1],
                        in_=bias[s, m0 : m0 + ms],
                    )

            for s in range(S):
                xts = xts_all[s]
                wts = wts_all[s]

                def win(ci, nt_idx, kh, kw, lo, hi):
                    xv = xts[ci].rearrange("p (h w) -> p h w", w=WP)
                    h0 = nt_idx * ROWS_PER_NT
                    return xv[lo:hi, h0 + kh : h0 + kh + ROWS_PER_NT,
                              kw : kw + W]

                for mi, (m0, ms) in enumerate(MCH):
                    for ng in range(NTILES // NG):
                        pa = [psum_pool.tile([128, NT], mybir.dt.float32,
                                             name=f"pa{j}", tag=f"pa{j}")
                              for j in range(NG)]
                        pb = [psum_pool.tile([128, NT], mybir.dt.float32,
                                             name=f"pb{j}", tag=f"pb{j}")
                              for j in range(NG)] if PACK_K else None

                        # full-K chunks
                        first = True
                        for ci in (0, 1):
                            c0, cs = KCH[ci]
                            for t in range(KK * KK):
                                kh, kw = t // KK, t % KK
                                lhsT = wts[ci][:cs, t * COUT + m0 :
                                               t * COUT + m0 + ms]
                                for j in range(NG):
                                    rhs = win(ci, ng * NG + j, kh, kw, 0, cs)
                                    nc.tensor.matmul(pa[j][:ms], lhsT, rhs,
                                                     start=first, stop=False)
                                first = False
                        # remainder chunk (64 rows)
                        c0, cs = KCH[2]
                        for kh in range(KK):
                            for kw in range(KK):
                                t = kh * KK + kw
                                col = t * COUT + m0
                                if PACK_K and kw == 1:
                                    continue  # folded into kw==0's row-B
                                lhsT = wts[2][:cs, col : col + ms]
                                for j in range(NG):
                                    rhs = win(2, ng * NG + j, kh, kw, 0, cs)
                                    last = (kh == KK - 1 and
                                            kw == KK - 1)
                                    nc.tensor.matmul(
                                        pa[j][:ms], lhsT, rhs,
                                        start=False, stop=last,
                                        tile_position=(0, 0))
                                    if PACK_K and kw == 0:
                                        lhsTb = wts[2][64 : 64 + cs,
                                                       col : col + ms]
                                        rhsb = win(2, ng * NG + j, kh, 0,
                                                   64, 64 + cs)
                                        nc.tensor.matmul(
                                            pb[j][:ms], lhsTb, rhsb,
                                            start=(kh == 0),
                                            stop=(kh == KK - 1),
                                            tile_position=(64, 0))

                        for j in range(NG):
                            nt_idx = ng * NG + j
                            ot = opool.tile([128, NT], mybir.dt.float32,
                                            name="ot", tag="ot")
                            bb = btile[:ms, s * 3 + mi : s * 3 + mi + 1]
                            nc.vector.tensor_scalar_add(ot[:ms], pa[j][:ms],
                                                        bb)
                            if PACK_K:
                                nc.vector.tensor_tensor(
                                    out=ot[:ms], in0=ot[:ms],
                                    in1=pb[j][:ms],
                                    op=mybir.AluOpType.add)
                            nc.scalar.dma_start(
                                out=y[s, m0 : m0 + ms,
                                      nt_idx * NT : (nt_idx + 1) * NT],
                                in_=ot[:ms],
                            )
    nc.finalize()
    return nc


def round_fp32r(a):
    """Round fp32 to the PE's fp32r format (11 mantissa bits, RNE).

    Idempotent under the hardware's own input rounding, so pre-rounding on
    the host changes nothing numerically vs letting the PE round."""
    if DT_MM != mybir.dt.float32r:
        return a
    b = a.view(np.uint32)
    r = (b + np.uint32(0x7FF) + ((b >> np.uint32(12)) & np.uint32(1))) \
        & np.uint32(0xFFFFF000)
    return r.view(np.float32)


def prep_inputs(features, weights, bias, class_id):
    f = np.asarray(features, dtype=np.float32)
    w = np.asarray(weights, dtype=np.float32)
    b = np.asarray(bias, dtype=np.float32)
    cid = np.asarray(class_id).astype(np.int64)

    xpad = np.zeros((B, CIN, HP, WP), np.float32)
    xpad[:, :, 1 : H + 1, 1 : W + 1] = round_fp32r(f)
    wsel = w[cid]                                   # [B, COUT, CIN, 3, 3]
    # lhsT layout: [tap, CIN, COUT]
    wt = round_fp32r(np.ascontiguousarray(
        wsel.transpose(0, 3, 4, 2, 1).reshape(B, KK * KK, CIN, COUT)))
    bsel = np.ascontiguousarray(b[cid])             # [B, COUT]

    in_maps = []
    for core in range(NCORES):
        sl = slice(core * S, (core + 1) * S)
        in_maps.append({
            "xpad": np.ascontiguousarray(xpad[sl].reshape(S, CIN, HP * WP)),
            "wt": wt[sl],
            "bias": bsel[sl],
        })
    return in_maps


def run(features, weights, bias, class_id, trace=False):
    in_maps = prep_inputs(features, weights, bias, class_id)
    nc = build_nc()
    last_exc = None
    for attempt in range(4):
        try:
            res = run_bass_kernel_spmd(nc, in_maps,
                                       core_ids=list(range(NCORES)),
                                       trace=trace)
            break
        except Exception as exc:  # transient device faults: retry
            last_exc = exc
            time.sleep(15 * (attempt + 1))
    else:
        raise last_exc
    out = np.concatenate(
        [r["y"].reshape(S, COUT, H, W) for r in res.results], axis=0)
    return out, res


def kernel(features, weights, bias, class_id):
    out, _ = run(features, weights, bias, class_id)
    return out


# revision 48
# speedup vs baseline: 1.0332x; 1.0332x over previous
"""MoE per-sample expert conv3x3 (320->320, 64x64, B=16, 5 experts) on 8 trn2 cores.

Strategy: data-parallel over batch (2 samples/core). Host gathers each
sample's expert weights (weights[class_id]), transposes them to lhsT layout
[tap, CIN, COUT], and zero-pads features to 66x66 so the conv becomes 9
shifted matmuls accumulating in PSUM (no boundary handling on device).

float32r matmuls: full fp32 I/O, 11-bit mantissa multiplies, 4x the fp32
rate on the PE at N>=256.

CIN=320 splits into chunks (128,128,64). The 64-row remainder chunk would
leave half the PE array rows idle, so its kw=0/kw=1 tap pairs are packed
via tile_position row tiling: rows 0:64 of the array run tap (kh,0) while
rows 64:128 concurrently run tap (kh,1), whose weights are overlaid in the
upper partition half of the chunk-2 weight tile and whose features come
from a +1-shifted duplicate in the upper half of the chunk-2 feature tile.
The row-B results accumulate in a second PSUM bank per n-tile and are
folded in during the bias epilogue.

DMA: features on the SP HWDGE ring (nc.sync), weights/bias/outputs on the
ACT ring (nc.scalar) so the two input streams load in parallel at startup.
"""

import time

import numpy as np

import concourse.bass as bass
import concourse.mybir as mybir
import concourse.tile as tile
from concourse import bacc
from concourse.bass_utils import run_bass_kernel_spmd

B = 16
NCORES = 8
S = B // NCORES          # samples per core
CIN = 320
COUT = 320
H = W = 64
KK = 3
HP = WP = H + 2          # padded spatial
NPIX = H * W             # 4096
NT = 512                 # output free-dim tile: 8 rows x 64 cols
ROWS_PER_NT = NT // W    # 8
NTILES = NPIX // NT      # 8
NG = 2                   # n-tiles per PSUM group
KCH = [(0, 128), (128, 128), (256, 64)]   # CIN chunks (k0, ksize)
MCH = [(0, 128), (128, 128), (256, 64)]   # COUT chunks (m0, msize)

DT_MM = mybir.dt.float32r   # matmul compute dtype (fp32 storage, fast path)
PACK_K = True               # row-pack the CIN=64 remainder chunk tap pairs


def build_nc():
    nc = bacc.Bacc(None, target_bir_lowering=False)
    xpad = nc.dram_tensor("xpad", [S, CIN, HP * WP], DT_MM,
                          kind="ExternalInput")
    wt = nc.dram_tensor("wt", [S, KK * KK, CIN, COUT], DT_MM,
                        kind="ExternalInput")
    bias = nc.dram_tensor("bias", [S, COUT], mybir.dt.float32,
                          kind="ExternalInput")
    y = nc.dram_tensor("y", [S, COUT, NPIX], mybir.dt.float32,
                       kind="ExternalOutput")

    with tile.TileContext(nc) as tc:
        with (
            tc.tile_pool(name="wpool", bufs=1) as wpool,
            tc.tile_pool(name="xpool", bufs=2) as xpool,
            tc.tile_pool(name="bpool", bufs=1) as bpool,
            tc.tile_pool(name="opool", bufs=8) as opool,
            tc.tile_pool(name="psum", bufs=2, space="PSUM") as psum_pool,
        ):
            btile = bpool.tile([128, S * 3], mybir.dt.float32, name="btile",
                               tag="btile")

            # The DMA pool services the SP and ACT HWDGE rings round-robin,
            # so issue loads on ALTERNATING rings in PE-consumption order —
            # round-robin delivery then matches the order the matmuls need
            # the data.
            # Sample 0's startup loads alternate rings in consumption
            # order so round-robin delivery matches the matmul order;
            # sample 1's loads go entirely on the SP ring so the ACT ring
            # stays clear for the epilogue out-DMAs (per-ring FIFO: outs
            # must not queue behind bulk input traffic).
            rings = [nc.sync, nc.scalar]
            rk = [0]

            def dma(out_ap, in_ap, s=0):
                if s == 0:
                    rings[rk[0] % 2].dma_start(out=out_ap, in_=in_ap)
                    rk[0] += 1
                else:
                    nc.sync.dma_start(out=out_ap, in_=in_ap)

            wts_all = {}
            xts_all = {}
            for s in range(S):
                # x in two row-pieces per chunk (Tile tracks subtile deps,
                # so the first n-tile groups unblock on the top piece)
                cut = 34 * WP
                xts = []
                wts = []
                for ci, (c0, cs) in enumerate(KCH):
                    xt = xpool.tile([128, HP * WP], DT_MM,
                                    name=f"x_{s}_{ci}", tag=f"x{ci}")
                    xts.append(xt)
                    wti = wpool.tile([128, KK * KK * COUT], DT_MM,
                                     name=f"wt_{s}_{ci}", tag=f"wt_{s}_{ci}")
                    wts.append(wti)
                for ci, (c0, cs) in enumerate(KCH):
                    dma(xts[ci][:cs, :cut], xpad[s, c0 : c0 + cs, :cut], s)
                    if PACK_K and ci == 2:
                        # upper half: same 64 channels shifted by +1 column,
                        # so row-tile B at window(kh,0) reads tap (kh,1) data
                        dma(xts[ci][64:128, 0 : cut],
                            xpad[s, c0 : c0 + cs, 1 : cut + 1], s)
                    dst = wts[ci][:cs].rearrange("c (t o) -> c t o", o=COUT)
                    src = wt[s].rearrange("t c o -> c t o")[c0 : c0 + cs]
                    # tap 0 first: unblocks the first accumulation slot
                    dma(dst[:, 0:1], src[:, 0:1], s)
                    dma(dst[:, 1:], src[:, 1:], s)
                    if PACK_K and ci == 2:
                        # upper half, at tap-(kh,0) columns: tap (kh,1)
                        # weights for the row-B halves of the k-pairs
                        for kh in range(KK):
                            dma(wts[ci][64 : 64 + cs,
                                        (kh * KK) * COUT :
                                        (kh * KK) * COUT + COUT],
                                wt[s, kh * KK + 1, c0 : c0 + cs], s)
                for ci, (c0, cs) in enumerate(KCH):
                    dma(xts[ci][:cs, cut:], xpad[s, c0 : c0 + cs, cut:], s)
                    if PACK_K and ci == 2:
                        dma(xts[ci][64:128, cut : HP * WP - 1],
                            xpad[s, c0 : c0 + cs, cut + 1 : HP * WP], s)
                xts_all[s] = xts
                wts_all[s] = wts

            for s in range(S):
                for mi, (m0, ms) in enumerate(MCH):
                    nc.scalar.dma_start(
                        out=btile[:ms, s * 3 + mi : s * 3 + mi + 1],
                        in_=bias[s, m0 : m0 + ms],
                    )

            for s in range(S):
                xts = xts_all[s]
                wts = wts_all[s]

                def win(ci, nt_idx, kh, kw, lo, hi):
                    xv = xts[ci].rearrange("p (h w) -> p h w", w=WP)
                    h0 = nt_idx * ROWS_PER_NT
                    return xv[lo:hi, h0 + kh : h0 + kh + ROWS_PER_NT,
                              kw : kw + W]

                for mi, (m0, ms) in enumerate(MCH):
                    for ng in range(NTILES // NG):
                        pa = [psum_pool.tile([128, NT], mybir.dt.float32,
                                             name=f"pa{j}", tag=f"pa{j}")
                              for j in range(NG)]
                        pb = [psum_pool.tile([128, NT], mybir.dt.float32,
                                             name=f"pb{j}", tag=f"pb{j}")
                              for j in range(NG)] if PACK_K else None

                        # full-K chunks
                        first = True
                        for ci in (0, 1):
                            c0, cs = KCH[ci]
                            for t in range(KK * KK):
                                kh, kw = t // KK, t % KK
                                lhsT = wts[ci][:cs, t * COUT + m0 :
                                               t * COUT + m0 + ms]
                                for j in range(NG):
                                    rhs = win(ci, ng * NG + j, kh, kw, 0, cs)
                                    nc.tensor.matmul(pa[j][:ms], lhsT, rhs,
                                                     start=first, stop=False)
                                first = False
                        # remainder chunk (64 rows)
                        c0, cs = KCH[2]
                        for kh in range(KK):
                            for kw in range(KK):
                                t = kh * KK + kw
                                col = t * COUT + m0
                                if PACK_K and kw == 1:
                                    continue  # folded into kw==0's row-B
                                lhsT = wts[2][:cs, col : col + ms]
                                for j in range(NG):
                                    rhs = win(2, ng * NG + j, kh, kw, 0, cs)
                                    last = (kh == KK - 1 and
                                            kw == KK - 1)
                                    nc.tensor.matmul(
                                        pa[j][:ms], lhsT, rhs,
                                        start=False, stop=last,
                                        tile_position=(0, 0))
                                    if PACK_K and kw == 0:
                                        lhsTb = wts[2][64 : 64 + cs,
                                                       col : col + ms]
                                        rhsb = win(2, ng * NG + j, kh, 0,
                                                   64, 64 + cs)
                                        nc.tensor.matmul(
                                            pb[j][:ms], lhsTb, rhsb,
                                            start=(kh == 0),
                                            stop=(kh == KK - 1),
                                            tile_position=(64, 0))

                        for j in range(NG):
                            nt_idx = ng * NG + j
                            ot = opool.tile([128, NT], mybir.dt.float32,
                                            name="ot", tag="ot")
                            bb = btile[:ms, s * 3 + mi : s * 3 + mi + 1]
                            nc.vector.tensor_scalar_add(ot[:ms], pa[j][:ms],
                                                        bb)
                            if PACK_K:
                                nc.vector.tensor_tensor(
                                    out=ot[:ms], in0=ot[:ms],
                                    in1=pb[j][:ms],
                                    op=mybir.AluOpType.add)
                            nc.scalar.dma_start(
                                out=y[s, m0 : m0 + ms,
                                      nt_idx * NT : (nt_idx + 1) * NT],
                                in_=ot[:ms],
                            )
    nc.finalize()
    return nc


def round_fp32r(a):
    """Round fp32 to the PE's fp32r format (11 mantissa bits, RNE).

    Idempotent under the hardware's own input rounding, so pre-rounding on
    the host changes nothing numerically vs letting the PE round."""
    if DT_MM != mybir.dt.float32r:
        return a
    b = a.view(np.uint32)
    r = (b + np.uint32(0x7FF) + ((b >> np.uint32(12)) & np.uint32(1))) \
        & np.uint32(0xFFFFF000)
    return r.view(np.float32)


def prep_inputs(features, weights, bias, class_id):
    f = np.asarray(features, dtype=np.float32)
    w = np.asarray(weights, dtype=np.float32)
    b = np.asarray(bias, dtype=np.float32)
    cid = np.asarray(class_id).astype(np.int64)

    xpad = np.zeros((B, CIN, HP, WP), np.float32)
    xpad[:, :, 1 : H + 1, 1 : W + 1] = round_fp32r(f)
    wsel = w[cid]                                   # [B, COUT, CIN, 3, 3]
    # lhsT layout: [tap, CIN, COUT]
    wt = round_fp32r(np.ascontiguousarray(
        wsel.transpose(0, 3, 4, 2, 1).reshape(B, KK * KK, CIN, COUT)))
    bsel = np.ascontiguousarray(b[cid])             # [B, COUT]

    in_maps = []
    for core in range(NCORES):
        sl = slice(core * S, (core + 1) * S)
        in_maps.append({
            "xpad": np.ascontiguousarray(xpad[sl].reshape(S, CIN, HP * WP)),
            "wt": wt[sl],
            "bias": bsel[sl],
        })
    return in_maps


def run(features, weights, bias, class_id, trace=False):
    in_maps = prep_inputs(features, weights, bias, class_id)
    nc = build_nc()
    last_exc = None
    for attempt in range(4):
        try:
            res = run_bass_kernel_spmd(nc, in_maps,
                                       core_ids=list(range(NCORES)),
                                       trace=trace)
            break
        except Exception as exc:  # transient device faults: retry
            last_exc = exc
            time.sleep(15 * (attempt + 1))
    else:
        raise last_exc
    out = np.concatenate(
        [r["y"].reshape(S, COUT, H, W) for r in res.results], axis=0)
    return out, res


def kernel(features, weights, bias, class_id):
    out, _ = run(features, weights, bias, class_id)
    return out
